# revision 29
# baseline (speedup 1.0000x reference)
"""CODA-Prompt forward kernel for 8 TRN2 NeuronCores (data-parallel over batch).

Reference computation (forward only; stop_gradient is identity):
    K = (task_count + 1) * 10            # active pool slice, all branches
    x_mean[b,d]  = mean_n x[b,n,d]
    aq[b,k]      = (x_mean . (att[k]*nK[k])) / max(||x_mean*att[k]||, eps)
    P_[b,l,d]    = sum_k aq[b,k] * prompt[k,l,d]
    out          = concat([P_, x], axis=1)            # [B, 8+197, 768]

Device kernel per core (B=32 of 256 batches).  The x shard is a flat
row array [6304, 768] streamed in [128, 4, 768] tiles of 512
consecutive rows (row r = 512t + 4p + u): every DMA spans all 128
partitions with 12 KB per-partition runs, so all 16 SDMA engines
carry equal load.  Nearly all tiles stay resident in SBUF (bufs=11),
so the in-stream never stalls on the out-stream; ins ride two HWDGE
queues (sync+scalar) while every out-copy piece rides the gpsimd
SWDGE queue (HWDGE HBM-writes stall the issuing engine ~1.8 us for
small pieces; SWDGE ~0.7 us).  With a 2:1 queue split the ins finish
well before the outs drain, so the sum/aq/P_ compute tail hides under
the remaining out-backlog.
Each tile is written back as per-batch row segments (out row = x row +
8(b+1)); segments start/end mid-quad at unaligned batch boundaries,
adding a <=3-row piece there.  Token sums: DVE folds each quad (2
adds), PE accumulates batch-on-partition in PSUM via per-tile fp32
membership matmuls (memb[p, t, b] = 1 iff quad (512t+4p..+3) starts in
batch b).  A quad straddling an unaligned boundary 197b is attributed
to b-1; the 48 spilled rows (3 strided gathers) are moved to the right
batch by one signed correction matmul.  Stage 2/3 computes aq and P_
and writes the P_ rows (disjoint from the copy rows) at the end.
Host combines the small pool tensors:
    attnkT[d,k] = att[k,d] * nK[k,d],  attn2T[d,k] = att[k,d]^2,
    prflat[k,:] = prompt[k].reshape(6144)
aq is scale-invariant in x_mean, so the 1/197 mean scaling cancels and
the kernel works with raw token sums.
"""

import numpy as np

TOP_K = 10
LENGTH = 8
EMBED_DIM = 768
N_TOK = 197
B_FULL = 256
N_CORES = 8
B = B_FULL // N_CORES          # 32 batches per core
PF = LENGTH * EMBED_DIM        # 6144 flattened prompt row
R = B * N_TOK                  # 6304 flat x rows per core
OROWS = B * (LENGTH + N_TOK)   # 6560 flat out rows per core
U = 8                          # rows folded per partition (also the
                               # scatter group: 8(b+1) % 8 == 0 keeps
                               # every clean group 8-aligned in out, and
                               # 24 KB per indirect descriptor doubles
                               # the Q7 emission-bound scatter rate)
TILE_R = 128 * U               # 1024 rows per tile
NT = (R + TILE_R - 1) // TILE_R    # 7 tiles (last has 160 rows)

_PROGRAMS = {}


def _tile_rows(t):
    return min(TILE_R, R - TILE_R * t)


def _tile_segs():
    """Per tile: list of (local_row_a, local_row_b, batch)."""
    segs = []
    for t in range(NT):
        r0 = TILE_R * t
        nr = _tile_rows(t)
        cuts = [r0]
        b0 = r0 // N_TOK
        nb = (r0 + nr - 1) // N_TOK
        for b in range(b0 + 1, nb + 1):
            cuts.append(N_TOK * b)
        cuts.append(r0 + nr)
        segs.append([(cuts[i] - r0, cuts[i + 1] - r0, b0 + i)
                     for i in range(len(cuts) - 1)])
    return segs


def _corr_layout():
    """Spilled rows: boundary 197b with m = 197b % U != 0 leaves the last
    U - m rows of its quad in batch b while memb counted them in b - 1.
    Grouped by m for strided gathering."""
    groups = []
    for m in range(1, U):
        bs = [b for b in range(1, B) if (N_TOK * b) % U == m]
        if bs:
            groups.append((m, len(bs), U - m, bs))
    total = sum(g[1] * g[2] for g in groups)
    return groups, total


def _build_program(K):
    import concourse.bacc as bacc
    import concourse.mybir as mybir
    import concourse.tile as tile
    import concourse.bass as bass
    from concourse.bass import ts
    from concourse.masks import make_identity

    f32 = mybir.dt.float32
    nc = bacc.Bacc()

    groups, ncorr = _corr_layout()

    x = nc.dram_tensor("x", [R, EMBED_DIM], f32, kind="ExternalInput")
    prflat = nc.dram_tensor("prflat", [K, PF], f32, kind="ExternalInput")
    attnkT = nc.dram_tensor("attnkT", [EMBED_DIM, K], f32, kind="ExternalInput")
    attn2T = nc.dram_tensor("attn2T", [EMBED_DIM, K], f32, kind="ExternalInput")
    memb = nc.dram_tensor("memb", [128, NT * B], f32, kind="ExternalInput")
    corrmat = nc.dram_tensor("corrmat", [ncorr, B], f32, kind="ExternalInput")
    oidx = nc.dram_tensor("oidx", [128, NT], mybir.dt.int32,
                          kind="ExternalInput")
    # out viewed in quad-rows (4 x 768) + one scratch quad-row that the
    # batch-boundary-straddling quads dump into (host slices it off)
    OQ = OROWS // U + 1
    out = nc.dram_tensor("out", [OQ, U * EMBED_DIM], f32,
                         kind="ExternalOutput")

    with tile.TileContext(nc) as tc:
        with (
            tc.tile_pool(name="const", bufs=1) as constp,
            tc.tile_pool(name="xt", bufs=5) as xtp,
            tc.tile_pool(name="xs", bufs=2) as xsp,
            tc.tile_pool(name="misc", bufs=1) as miscp,
            tc.tile_pool(name="psb", bufs=2) as psbp,
            tc.tile_pool(name="psA", bufs=1, space="PSUM") as psap,
            tc.tile_pool(name="pst", bufs=1, space="PSUM") as pstp,
            tc.tile_pool(name="pp", bufs=2, space="PSUM") as ppp,
            tc.tile_pool(name="pt", bufs=2, space="PSUM") as ptp,
        ):
            # --- constants (gpsimd queue) ----------------------------------
            ident = constp.tile([128, 128], f32)
            make_identity(nc, ident)
            prflat_sb = constp.tile([K, PF], f32)
            nc.gpsimd.dma_start(out=prflat_sb, in_=prflat[:, :])
            attnkT_sb = constp.tile([128, 6, K], f32)
            nc.gpsimd.dma_start(
                out=attnkT_sb,
                in_=attnkT[:, :].rearrange("(c p) k -> p c k", p=128))
            attn2T_sb = constp.tile([128, 6, K], f32)
            nc.gpsimd.dma_start(
                out=attn2T_sb,
                in_=attn2T[:, :].rearrange("(c p) k -> p c k", p=128))
            memb_sb = constp.tile([128, NT, B], f32)
            nc.gpsimd.dma_start(
                out=memb_sb,
                in_=memb[:, :].rearrange("p (t b) -> p t b", b=B))
            corrmat_sb = constp.tile([ncorr, B], f32)
            nc.gpsimd.dma_start(out=corrmat_sb, in_=corrmat[:, :])
            oidx_sb = constp.tile([128, NT], mybir.dt.int32)
            nc.gpsimd.dma_start(out=oidx_sb, in_=oidx[:, :])
            # Preheat: have PE consume each constant once so no later matmul
            # needs >1 semaphore wait.
            scr = ptp.tile([1, 1], f32, tag="pt", name="scr")
            for ap_ in (ident[:1, :1], attnkT_sb[:1, 0, :1],
                        attn2T_sb[:1, 0, :1], prflat_sb[:1, :1],
                        memb_sb[:1, 0, :1], corrmat_sb[:1, :1]):
                nc.tensor.matmul(scr, ap_, ap_, start=True, stop=True)

            # token-quad sums, batch-on-partition, 2 psum halves
            psum_h = [psap.tile([B, 384], f32, tag=f"psum{h}", name=f"psum{h}")
                      for h in range(2)]

            # --- stage 1: stream x, copy out, fold quads, accumulate -------
            # The bulk of the out-copy is NT indirect quad-scatters (one
            # per tile): partition p's 12 KB quad payload lands at out
            # quad-row oidx[p, t] (clean quads are always 4-aligned in out
            # since 8(b+1) % 4 == 0); quads straddling an unaligned batch
            # boundary dump into the scratch quad-row and their rows are
            # written by two small pieces instead.  This replaces the ~92
            # per-segment pieces whose issue cost and completion-semaphore
            # convoys starved the DMA rings.
            def flat_rows(ro, nrows):
                return bass.AP(tensor=out[:, :].tensor,
                               offset=ro * EMBED_DIM,
                               ap=[[EMBED_DIM, nrows], [1, EMBED_DIM]])

            piece_rr = [0]

            D2D = (5,)

            # small last tile first: its in-DMA lands in ~8us, so the
            # serial Q7 scatter-emission chain starts ~30us earlier; the
            # D2D tile next (its in-DMA only feeds the sums, and loading
            # it early un-gates the aq/P_ tail so it hides under the
            # scatter chain)
            order = [NT - 1] + list(D2D) + [t for t in range(NT - 1)
                                            if t not in D2D]
            for t in order:
                first = t == NT - 1
                r0 = TILE_R * t
                nr = _tile_rows(t)
                nq = nr // U
                # flat [p, 4*768] payload: the HW indirect-DMA ucode
                # requires a single free dim on the SBUF side
                xt = xtp.tile([128, U * EMBED_DIM], f32)
                in_eng = nc.sync if t % 2 == 0 else nc.scalar
                in_eng.dma_start(
                    out=xt[:nq],
                    in_=x[r0:r0 + nr, :].rearrange("(p u) d -> p (u d)", u=U))
                if t not in D2D:
                    nc.gpsimd.indirect_dma_start(
                        out=out[:, :],
                        out_offset=bass.IndirectOffsetOnAxis(
                            ap=oidx_sb[:nq, t:t + 1], axis=0),
                        in_=xt[:nq, :],
                        in_offset=None)

                # fold the 8-row group on DVE -> 1/8 PE streaming
                xs = xsp.tile([128, EMBED_DIM], f32)
                nc.vector.tensor_add(xs[:nq], xt[:nq, ts(0, EMBED_DIM)],
                                     xt[:nq, ts(1, EMBED_DIM)])
                for j in range(2, U):
                    nc.vector.tensor_add(xs[:nq], xs[:nq],
                                         xt[:nq, ts(j, EMBED_DIM)])
                for h in range(2):
                    nc.tensor.matmul(
                        psum_h[h],
                        memb_sb[:nq, t, :], xs[:nq, ts(h, 384)],
                        start=first, stop=False)

            # D2D tiles: copied x->out directly in DRAM by per-segment
            # pieces on the HWDGE queues.  These have no SBUF dependency,
            # so they stream from t=0 without convoying anything (their
            # bytes are read twice from HBM, but the queues are otherwise
            # idle once the ins are done).
            # straddling groups (boundary 197b with 197b % U != 0): their
            # left/right parts go out at different shifts.  Sourced from x
            # in DRAM (dep-free D2D), so they stream on the HWDGE queues
            # without stalling the in-DMAs or referencing tiles.
            for b in range(1, B):
                rb = N_TOK * b
                m = rb % U
                if m == 0:
                    continue
                rq = rb - m
                if rq // TILE_R in D2D:
                    continue
                for (ra, n_, shift) in ((rq, m, LENGTH * b),
                                        (rb, U - m, LENGTH * (b + 1))):
                    piece_rr[0] += 1
                    eng = nc.sync if piece_rr[0] % 2 == 0 else nc.scalar
                    eng.dma_start(
                        out=flat_rows(ra + shift, n_),
                        in_=bass.AP(tensor=x[:, :].tensor,
                                    offset=ra * EMBED_DIM,
                                    ap=[[EMBED_DIM, n_], [1, EMBED_DIM]]))
            for t in D2D:
                r0 = TILE_R * t
                for (la, lb, b) in _tile_segs()[t]:
                    piece_rr[0] += 1
                    eng = nc.sync if piece_rr[0] % 2 == 0 else nc.scalar
                    eng.dma_start(
                        out=flat_rows(r0 + la + LENGTH * (b + 1), lb - la),
                        in_=bass.AP(tensor=x[:, :].tensor,
                                    offset=(r0 + la) * EMBED_DIM,
                                    ap=[[EMBED_DIM, lb - la],
                                        [1, EMBED_DIM]]))

            # spilled rows x[197b .. 197b + (U-m)) grouped by m (sync;
            # gathered late to keep the early DMA-semaphore lanes clean)
            corr_sb = constp.tile([ncorr, EMBED_DIM], f32)
            row0 = 0
            for (m, nb, rows, bs) in groups:
                stride_b = (bs[1] - bs[0]) * N_TOK if nb > 1 else 1
                ap = [[stride_b * EMBED_DIM, nb], [EMBED_DIM, rows],
                      [1, EMBED_DIM]]
                if rows == 1:
                    ap = [[stride_b * EMBED_DIM, nb], [1, EMBED_DIM]]
                nc.sync.dma_start(
                    out=corr_sb[row0:row0 + nb * rows],
                    in_=bass.AP(tensor=x[:, :].tensor,
                                offset=N_TOK * bs[0] * EMBED_DIM, ap=ap))
                row0 += nb * rows

            # move spilled rows to their true batch (signed matmul)
            for h in range(2):
                nc.tensor.matmul(psum_h[h], corrmat_sb, corr_sb[:, ts(h, 384)],
                                 start=False, stop=True)

            # --- stage 2: copy sums, transpose, numer/norm2, aq ------------
            means = miscp.tile([B, EMBED_DIM], f32)
            for h in range(2):
                nc.vector.tensor_copy(means[:, ts(h, 384)], psum_h[h])

            meansT = miscp.tile([128, 6, B], f32)
            for j in range(6):
                pt = ptp.tile([128, B], f32)
                nc.tensor.transpose(pt, means[:, ts(j, 128)], ident[:B, :B])
                nc.vector.tensor_copy(meansT[:, j, :], pt)
            sqT = miscp.tile([128, 6, B], f32)
            nc.vector.tensor_mul(sqT, meansT, meansT)

            pn = pstp.tile([K, B], f32)
            pq = pstp.tile([K, B], f32)
            for j in range(6):
                nc.tensor.matmul(pn, attnkT_sb[:, j, :], meansT[:, j, :],
                                 start=(j == 0), stop=(j == 5))
            for j in range(6):
                nc.tensor.matmul(pq, attn2T_sb[:, j, :], sqT[:, j, :],
                                 start=(j == 0), stop=(j == 5))

            denom = miscp.tile([K, B], f32)
            nc.scalar.sqrt(denom, pq)
            nc.vector.tensor_scalar_max(denom, denom, 1e-12)
            recip = miscp.tile([K, B], f32)
            nc.vector.reciprocal(recip, denom)
            aqT = miscp.tile([K, B], f32)
            nc.vector.tensor_mul(aqT, pn, recip)

            # --- stage 3: P_ = aq @ prflat, in 4 chunks of 2 P_ rows -------
            for c in range(4):
                p_sb = psbp.tile([B, PF // 4], f32)
                for h in range(4):
                    pp = ppp.tile([B, 384], f32)
                    nc.tensor.matmul(pp, aqT,
                                     prflat_sb[:, ts(4 * c + h, 384)],
                                     start=True, stop=True)
                    nc.vector.tensor_copy(p_sb[:, ts(h, 384)], pp)
                nc.scalar.dma_start(
                    out=bass.AP(
                        tensor=out[:, :].tensor,
                        offset=2 * c * EMBED_DIM,
                        ap=[[(LENGTH + N_TOK) * EMBED_DIM, B],
                            [EMBED_DIM, 2], [1, EMBED_DIM]]),
                    in_=p_sb.rearrange("p (l d) -> p l d", l=2))

    nc.finalize()
    return nc


def _host_prep(prompt, attention, prompt_key, task_count):
    K = (int(task_count) + 1) * TOP_K
    pk = np.asarray(prompt_key[:K], dtype=np.float32)
    att = np.asarray(attention[:K], dtype=np.float32)
    pr = np.asarray(prompt[:K], dtype=np.float32)
    nrm = np.sqrt(np.sum(pk * pk, axis=1, keepdims=True, dtype=np.float32))
    nK = pk / np.maximum(nrm, np.float32(1e-12))
    attnkT = np.ascontiguousarray((att * nK).T)
    attn2T = np.ascontiguousarray((att * att).T)
    prflat = np.ascontiguousarray(pr.reshape(K, PF))
    return K, attnkT, attn2T, prflat


def _make_memb():
    """[128, NT*B] f32: quad p of tile t -> batch of its first row."""
    memb = np.zeros((128, NT * B), dtype=np.float32)
    for t in range(NT):
        r0 = TILE_R * t
        for p in range(_tile_rows(t) // U):
            memb[p, t * B + (r0 + U * p) // N_TOK] = 1.0
    return memb


def _make_oidx():
    """[128, NT] i32: out quad-row for the quad at rows [512t+4p, +4), or
    the scratch quad-row if the quad straddles an unaligned boundary."""
    scratch = OROWS // U
    oidx = np.full((128, NT), scratch, dtype=np.int32)
    for t in range(NT):
        r0 = TILE_R * t
        for p in range(_tile_rows(t) // U):
            r = r0 + U * p
            b = r // N_TOK
            if (r + U - 1) // N_TOK == b:
                oidx[p, t] = (r + LENGTH * (b + 1)) // U
    return oidx


def _make_corrmat():
    """[ncorr, B]: spilled row i (in batch b, counted in b-1): +1/-1."""
    groups, ncorr = _corr_layout()
    m = np.zeros((ncorr, B), dtype=np.float32)
    i = 0
    for (_, nb, rows, bs) in groups:
        for b in bs:
            for _r in range(rows):
                m[i, b] = 1.0
                m[i, b - 1] = -1.0
                i += 1
    return m


def _shard_x(x_embed, i):
    return np.ascontiguousarray(
        x_embed[i * B:(i + 1) * B].reshape(R, EMBED_DIM))


def kernel(x_embed, prompt, attention, prompt_key, iseval, task_count,
           _want_trace=False, **_trace_kwargs):
    from concourse.bass_utils import run_bass_kernel_spmd

    x_embed = np.asarray(x_embed, dtype=np.float32)
    assert x_embed.shape == (B_FULL, N_TOK, EMBED_DIM)
    K, attnkT, attn2T, prflat = _host_prep(prompt, attention, prompt_key,
                                           task_count)

    if K not in _PROGRAMS:
        _PROGRAMS[K] = _build_program(K)
    nc = _PROGRAMS[K]

    memb = _make_memb()
    corrmat = _make_corrmat()
    oidx = _make_oidx()
    in_maps = []
    for i in range(N_CORES):
        in_maps.append({
            "x": _shard_x(x_embed, i),
            "prflat": prflat,
            "attnkT": attnkT,
            "attn2T": attn2T,
            "memb": memb,
            "corrmat": corrmat,
            "oidx": oidx,
        })
    res = run_bass_kernel_spmd(nc, in_maps, core_ids=list(range(N_CORES)),
                               trace=_want_trace, **_trace_kwargs)
    full = np.concatenate(
        [res.results[i]["out"].reshape(-1, EMBED_DIM)[:OROWS].reshape(
            B, LENGTH + N_TOK, EMBED_DIM) for i in range(N_CORES)],
        axis=0)
    if _want_trace:
        return full, res
    return full
